# revision 12
# baseline (speedup 1.0000x reference)
"""Dilated attention Trainium2 kernel (8 NeuronCores, SPMD).

Sharding: batch (2) x head-group (4 groups of 4 heads) -> 8 cores.
Each core computes, for its batch b and head group g:
    q^T = (Wq_g^T @ x_b^T),  k^T/v from dilated tokens,
    p^T = exp(k^T-block @ q^T-block / 8)  (scores transposed: ktok on partitions),
    u_aug = p @ [v | 1]  -> unnormalized context + softmax denominators,
    ctx^T = u^T * (1/r),  partial_out = ctx @ Wo_g-rows.
Host sums the 4 per-group partial outputs per batch and adds bo.
"""

import numpy as np

# ---- problem constants (hardcoded per contest rules) ----
B, S, E = 2, 4096, 1024
H, D = 16, 64
DIL = 4
SK = S // DIL          # 1024 dilated keys
NCORES = 8
GROUPS = 4             # head groups (cores per batch)
HPG = H // GROUPS      # 4 heads per core
CG = HPG * D           # 256 projected cols per core
SCALE = 1.0 / float(np.sqrt(D))

ET = E // 128          # 8 contraction tiles
M2 = CG // 128         # 2 col tiles
KT = SK // 128         # 8 ktok tiles
QBLKS = [(0, 1536), (1536, 1536), (3072, 1024)]  # qtok blocks (psum-bank friendly)

# normalize via DVE partition-broadcast (fast path); fallback uses a PE
# outer-product broadcast + extra eviction
DVE_BCAST = True

_CACHE = {}


def _build_program():
    import concourse.mybir as mybir
    import concourse.tile as tile
    from concourse import bacc
    from concourse.masks import make_identity

    f32 = mybir.dt.float32
    f32r = mybir.dt.float32r
    bf16 = mybir.dt.bfloat16
    EXP = mybir.ActivationFunctionType.Exp

    nc = bacc.Bacc(None, target_bir_lowering=False)

    x_d = nc.dram_tensor("x", [S, E], f32, kind="ExternalInput")
    wq_d = nc.dram_tensor("wq", [E, CG], f32, kind="ExternalInput")
    wk_d = nc.dram_tensor("wk", [E, CG], f32, kind="ExternalInput")
    wv_d = nc.dram_tensor("wv", [E, CG], f32, kind="ExternalInput")
    wo_d = nc.dram_tensor("wo", [CG, E], f32, kind="ExternalInput")
    bq_d = nc.dram_tensor("bq", [CG], f32, kind="ExternalInput")
    bk_d = nc.dram_tensor("bk", [CG], f32, kind="ExternalInput")
    bv_d = nc.dram_tensor("bv", [CG], f32, kind="ExternalInput")
    out_d = nc.dram_tensor("out", [S, E], f32, kind="ExternalOutput")

    def r32(ap):
        return ap.bitcast(f32r)

    with tile.TileContext(nc) as tc:
        with tc.tile_pool(name="const", bufs=1) as constp, \
             tc.tile_pool(name="qTp", bufs=1) as qTp, \
             tc.tile_pool(name="kTp", bufs=1) as kTp, \
             tc.tile_pool(name="vp", bufs=1) as vp:

            ident = constp.tile([128, 128], f32)
            make_identity(nc, ident)
            ones_f32 = constp.tile([1, 128], f32)
            nc.any.memset(ones_f32, 1.0)
            ones_row = constp.tile([1, 128], f32r)
            nc.vector.tensor_copy(ones_row, ones_f32)
            bq_sb = constp.tile([128, M2], f32)
            nc.sync.dma_start(bq_sb, bq_d[:].rearrange("(m p) -> p m", p=128))
            bk_sb = constp.tile([128, M2], f32)
            nc.sync.dma_start(bk_sb, bk_d[:].rearrange("(m p) -> p m", p=128))
            bv_sb = constp.tile([1, CG], f32r)
            nc.sync.dma_start(bv_sb, bv_d[:].unsqueeze(0).bitcast(f32r))

            qT = qTp.tile([128, M2, S], f32r)
            kT = kTp.tile([128, M2, SK], f32r)
            vaug = vp.tile([128, KT, HPG, D + 1], bf16)
            nc.any.memset(vaug[:, :, :, D:D + 1], 1.0)

            # ---------------- phase 1: x^T, projections ----------------
            with tc.tile_pool(name="xTp", bufs=1) as xTp, \
                 tc.tile_pool(name="xsp", bufs=2) as xsp, \
                 tc.tile_pool(name="wp", bufs=2) as wp, \
                 tc.tile_pool(name="tpp", bufs=2, space="PSUM") as tpp, \
                 tc.tile_pool(name="qpp", bufs=2, space="PSUM") as qpp:

                xT = xTp.tile([128, ET, S], f32r)
                for st in range(S // 128):
                    xs = xsp.tile([128, E], f32, tag="xs")
                    nc.sync.dma_start(xs, x_d[st * 128:(st + 1) * 128, :])
                    for eg in range(2):
                        tp = tpp.tile([128, 4, 128], f32, tag="tp")
                        for j in range(4):
                            e = eg * 4 + j
                            nc.tensor.transpose(
                                tp[:, j, :], xs[:, e * 128:(e + 1) * 128], ident)
                        nc.vector.tensor_copy(
                            xT[:, eg * 4:eg * 4 + 4, st * 128:(st + 1) * 128], tp)

                wq_sb = wp.tile([128, ET, CG], f32r, tag="w")
                nc.sync.dma_start(wq_sb, wq_d[:].rearrange("(k p) c -> p k c", p=128).bitcast(f32r))
                wk_sb = wp.tile([128, ET, CG], f32r, tag="w")
                nc.sync.dma_start(wk_sb, wk_d[:].rearrange("(k p) c -> p k c", p=128).bitcast(f32r))

                def xdil(k):
                    return xT[:, k, :].rearrange("p (n f) -> p n f", f=DIL)[:, :, 0]

                # Q projection: qT[cols, qtok]
                for m in range(M2):
                    for nb in range(S // 512):
                        qp = qpp.tile([128, 512], f32, tag="qp")
                        for k in range(ET):
                            nc.tensor.matmul(
                                qp, lhsT=wq_sb[:, k, m * 128:(m + 1) * 128],
                                rhs=xT[:, k, nb * 512:(nb + 1) * 512],
                                start=(k == 0), stop=(k == ET - 1))
                        nc.vector.tensor_scalar_add(
                            qT[:, m, nb * 512:(nb + 1) * 512], qp, bq_sb[:, m:m + 1])

                # K projection on dilated tokens: kT[cols, ktok]
                for m in range(M2):
                    for nb in range(SK // 512):
                        kp = qpp.tile([128, 512], f32, tag="qp")
                        for k in range(ET):
                            nc.tensor.matmul(
                                kp, lhsT=wk_sb[:, k, m * 128:(m + 1) * 128],
                                rhs=r32(xdil(k)[:, nb * 512:(nb + 1) * 512]),
                                start=(k == 0), stop=(k == ET - 1))
                        nc.vector.tensor_scalar_add(
                            kT[:, m, nb * 512:(nb + 1) * 512], kp, bk_sb[:, m:m + 1])

                # V projection: v[ktok, cols] (+1.0 ones column), bf16
                wv_sb = wp.tile([128, ET, CG], f32r, tag="w")
                nc.sync.dma_start(wv_sb, wv_d[:].rearrange("(k p) c -> p k c", p=128).bitcast(f32r))
                for mt in range(KT):
                    vps = qpp.tile([128, CG], f32, tag="qp")
                    for k in range(ET):
                        nc.tensor.matmul(
                            vps, lhsT=r32(xdil(k)[:, mt * 128:(mt + 1) * 128]),
                            rhs=wv_sb[:, k, :],
                            start=(k == 0), stop=False)
                    nc.tensor.matmul(
                        vps, lhsT=ones_row, rhs=bv_sb,
                        start=False, stop=True)
                    nc.vector.tensor_copy(
                        vaug[:, mt, :, 0:D],
                        vps.rearrange("p (h d) -> p h d", d=D))

            # ---------------- phase 2+3: attention + output proj ----------------
            with tc.tile_pool(name="pTp", bufs=1) as pTp, \
                 tc.tile_pool(name="ctxp", bufs=1) as ctxp, \
                 tc.tile_pool(name="wop", bufs=1) as wop, \
                 tc.tile_pool(name="rcpp", bufs=2) as rcpp, \
                 tc.tile_pool(name="utp", bufs=2) as utp, \
                 tc.tile_pool(name="osbp", bufs=3) as osbp, \
                 tc.tile_pool(name="spp", bufs=2, space="PSUM") as spp, \
                 tc.tile_pool(name="upp", bufs=2, space="PSUM") as upp:

                wo_sb = wop.tile([128, M2, E], f32r)
                nc.sync.dma_start(wo_sb, wo_d[:].rearrange("(k p) e -> p k e", p=128).bitcast(f32r))
                ctxT = ctxp.tile([128, M2, S], f32r)

                for (bo, bw) in QBLKS:
                    nw = bw // 512
                    for pair in range(2):
                        pTa = pTp.tile([128, KT, 1536], bf16, tag="pTa")
                        pTb = pTp.tile([128, KT, 1536], bf16, tag="pTb")
                        for mt in range(KT):
                            spa = spp.tile([128, 3, 512], f32, tag="sp")
                            spb = spp.tile([128, 3, 512], f32, tag="sp")
                            for n in range(nw):
                                qs = qT[:, pair, bo + n * 512: bo + (n + 1) * 512]
                                ks = kT[:, pair, mt * 128:(mt + 1) * 128]
                                nc.tensor.matmul(
                                    spa[:, n, :], lhsT=ks[0:64, :],
                                    rhs=qs[0:64, :], start=True, stop=True)
                                nc.tensor.matmul(
                                    spb[:, n, :], lhsT=ks[64:128, :],
                                    rhs=qs[64:128, :], start=True, stop=True)
                            nc.scalar.activation(
                                pTa[:, mt, 0:bw],
                                spa[:, 0:nw, :].rearrange("p a b -> p (a b)"),
                                EXP, scale=SCALE)
                            nc.scalar.activation(
                                pTb[:, mt, 0:bw],
                                spb[:, 0:nw, :].rearrange("p a b -> p (a b)"),
                                EXP, scale=SCALE)
                        for hl, pT_h in ((2 * pair, pTa), (2 * pair + 1, pTb)):
                            for nt in range(nw):
                                up = upp.tile([65, 512], f32, tag="up")
                                for mt in range(KT):
                                    nc.tensor.matmul(
                                        up, lhsT=vaug[:, mt, hl, :],
                                        rhs=pT_h[:, mt, nt * 512:(nt + 1) * 512],
                                        start=(mt == 0), stop=(mt == KT - 1))
                                rcp = rcpp.tile([1, 512], f32r, tag="rcp")
                                with nc.allow_low_precision(reason="fp32r normalize"):
                                    nc.vector.reciprocal(rcp, up[64:65, :])
                                bc = upp.tile([64, 512], f32, tag="up")
                                nc.tensor.matmul(
                                    bc, lhsT=ones_row[0:1, 0:64],
                                    rhs=rcp, start=True, stop=True)
                                ut = utp.tile([64, 512], f32, tag="ut")
                                nc.vector.tensor_copy(ut, up[0:64, :])
                                dst = ctxT[64 * (hl % 2):64 * (hl % 2) + 64,
                                           hl // 2,
                                           bo + nt * 512: bo + (nt + 1) * 512]
                                nc.vector.tensor_mul(dst, ut, bc)

                    # output projection for this qtok block
                    for m in range(bw // 128):
                        op = spp.tile([128, 2, 512], f32, tag="sp")
                        for n in range(2):
                            for k2 in range(M2):
                                nc.tensor.matmul(
                                    op[:, n, :],
                                    lhsT=ctxT[:, k2, bo + m * 128: bo + (m + 1) * 128],
                                    rhs=wo_sb[:, k2, n * 512:(n + 1) * 512],
                                    start=(k2 == 0), stop=(k2 == M2 - 1))
                        osb = osbp.tile([128, 2, 512], f32, tag="osb")
                        nc.vector.tensor_copy(osb, op)
                        nc.sync.dma_start(
                            out_d[bo + m * 128: bo + (m + 1) * 128, :],
                            osb.rearrange("p a b -> p (a b)"))

    nc.compile()
    return nc


def _get_program():
    if "nc" not in _CACHE:
        _CACHE["nc"] = _build_program()
    return _CACHE["nc"]


def make_in_maps(x, Wq, bq, Wk, bk, Wv, bv, Wo, bo):
    asf = np.ascontiguousarray
    in_maps = []
    for c in range(NCORES):
        b, g = c // GROUPS, c % GROUPS
        cs = slice(g * CG, (g + 1) * CG)
        in_maps.append({
            "x": asf(x[b], dtype=np.float32),
            "wq": asf(Wq[:, cs], dtype=np.float32),
            "wk": asf(Wk[:, cs], dtype=np.float32),
            "wv": asf(Wv[:, cs], dtype=np.float32),
            "wo": asf(Wo[cs, :], dtype=np.float32),
            "bq": asf(bq[cs], dtype=np.float32),
            "bk": asf(bk[cs], dtype=np.float32),
            "bv": asf(bv[cs], dtype=np.float32),
        })
    return in_maps


def gather_output(results, bo):
    out = np.zeros((B, S, E), dtype=np.float32)
    for c in range(NCORES):
        b = c // GROUPS
        out[b] += results[c]["out"]
    out += np.asarray(bo, dtype=np.float32)
    return out


def kernel(x, Wq, bq, Wk, bk, Wv, bv, Wo, bo, _trace=False):
    from concourse import bass_utils

    nc = _get_program()
    in_maps = make_in_maps(x, Wq, bq, Wk, bk, Wv, bv, Wo, bo)
    res = bass_utils.run_bass_kernel_spmd(
        nc, in_maps, core_ids=list(range(NCORES)), trace=_trace)
    _CACHE["last_result"] = res
    return gather_output(res.results, bo)
